# revision 5
# baseline (speedup 1.0000x reference)
"""Trainium2 Bass kernel for nn_CustomGCN (2-layer edge-sigmoid GCN).

Strategy (8 NeuronCores, SPMD):
  - Nodes are sharded across the 8 cores (12500 each, padded to 12544).
    Within a core, nodes are sorted by (in-)degree so that 128-node tiles
    have near-uniform degree (ELL padding stays small).
  - Launch 1: h1 = x @ W1 + b1, node-sharded. Host supplies x^T slices,
    PE computes h1^T tiles, host reassembles the row-major h1 table.
  - Launch 2 (conv1): per 128-node tile, per neighbor slot j, an indirect
    DMA gathers h1[src] rows (64B descriptors); DVE forms the per-edge
    dot with the tile's own h1 rows, ACT applies sigmoid and accumulates
    the per-node sum, pad slots (pointing at a zero row, sigmoid(0)=0.5)
    are corrected exactly via the true neighbor counts. u = relu(mean).
  - Launch 3 (conv2): h2 = u @ W2 + b2 gives <h2_d, h2_s> =
    a*u_d*u_s + b*(u_d+u_s) + c with a=|W2|^2, b=<W2,b2>, c=|b2|^2
    (computed on device), so only the scalar u[src] is gathered per edge.
    The final log_softmax over a single-column axis is v - logsumexp(v)
    = v - v, computed as such.

All arithmetic on tensor values happens on device; the host only does
structural preprocessing (edge sorting/packing, permutations, concat).
"""
import os
import sys
import types
import numpy as np

sys.path.insert(0, "/opt/trn_rl_repo")

# ---------------------------------------------------------------------------
# Environment compat: NTFF profile hook registry (enables trace=True under
# axon) and walrus sync-wait-limit workarounds.
# ---------------------------------------------------------------------------
if "antenv.axon_hooks" not in sys.modules:
    try:
        import antenv.axon_hooks  # noqa: F401
    except ImportError:
        _m = types.ModuleType("antenv.axon_hooks")
        _m._hook = None

        def _set(h):
            _m._hook = h

        def _get():
            return _m._hook

        _m.set_axon_ntff_profile_hook = _set
        _m.get_axon_ntff_profile_hook = _get
        sys.modules["antenv.axon_hooks"] = _m

import bass_rust
import concourse.bass as bass
import concourse.mybir as mybir
import concourse.tile as tile_mod
from concourse import bass_utils
from concourse.bass_utils import run_bass_kernel_spmd
from concourse.tile import TileContext
from concourse.vector_clock import ScopedClock

bass_utils.upload_artifacts = lambda tmpdir: f"local:{tmpdir}"

_MAXW = 1
_carrier_counter = [0]


def _split_inst_waits(inst):
    si = getattr(inst, "sync_info", None)
    if si is None:
        return []
    waits = list(si.on_wait)
    if len(waits) <= _MAXW:
        return []
    keep = waits[-_MAXW:]
    extra = waits[:-_MAXW]
    inst.sync_info = bass_rust.SyncInfo(on_update=list(si.on_update), on_wait=keep)
    carriers = []
    for j in range(0, len(extra), _MAXW):
        _carrier_counter[0] += 1
        carriers.append(
            mybir.InstNoOp(
                name=f"{inst.name}_wc{_carrier_counter[0]}",
                engine=inst.engine,
                sync_info=bass_rust.SyncInfo(
                    on_update=[], on_wait=extra[j : j + _MAXW]
                ),
                bass_nofuse=True,
            )
        )
    return carriers


_orig_postorder = tile_mod.postorder_instruction_blocks


def _patched_postorder(instructions, start_bb, output):
    ret = _orig_postorder(instructions, start_bb, output)
    for bb_name in list(output.keys()):
        new_list = []
        for inst in output[bb_name]:
            if isinstance(inst, (tile_mod.TileBranchInst, tile_mod.BassTileLoopBlock)):
                new_list.append(inst)
                continue
            new_list.extend(_split_inst_waits(inst))
            new_list.append(inst)
        output[bb_name][:] = new_list
    return output if ret is output else ret


def _patched_drain_and_barrier(self, tick_clock, wait_clock):
    drain_inst = self.nc.sync.drain()
    wait_clock.add_sem_waits(
        drain_inst.ins, ScopedClock({None: tick_clock.global_clock})
    )
    si = drain_inst.ins.sync_info
    if si is not None and len(si.on_wait) > _MAXW:
        waits = list(si.on_wait)
        drain_inst.ins.sync_info = bass_rust.SyncInfo(
            on_update=list(si.on_update), on_wait=waits[:_MAXW]
        )
        for i in range(_MAXW, len(waits), _MAXW):
            nd = self.nc.sync.drain()
            nd.ins.sync_info = bass_rust.SyncInfo(
                on_update=[], on_wait=waits[i : i + _MAXW]
            )

    self.nc.all_engine_barrier()
    assert self.sems is not None
    popped = self.nc._tile_sem_poison_stack.pop()
    assert popped is self._sem_poison
    self.nc.clear_and_free_semaphores(list(self.sems.allocated().values()))
    self.nc.all_engine_barrier()


if getattr(tile_mod.postorder_instruction_blocks, "__name__", "") != "_patched_postorder":
    tile_mod.postorder_instruction_blocks = _patched_postorder
    tile_mod.TileContext._drain_and_barrier = _patched_drain_and_barrier

# ---------------------------------------------------------------------------
# Problem constants
# ---------------------------------------------------------------------------
N = 100000
IN_C = 512
HID = 16
OUT_C = 64
NCORES = 8
P = 128
NPC = N // NCORES            # real nodes per core: 12500
TPC = 98                     # node tiles per core
NPC_PAD = TPC * P            # 12544
NTAB = NCORES * NPC_PAD      # 100352 table rows (permuted node space)
ZROW = NTAB                  # first zero row (pad-slot gather target)
NTAB_FULL = NTAB + P         # table with zero rows appended

F32 = mybir.dt.float32
I32 = mybir.dt.int32

EXEC_TIMES = {}              # launch name -> exec_time_ns (when tracing)
_TRACE = bool(int(os.environ.get("TRN_TRACE", "0")))


def _run(name, nc, in_maps):
    res = run_bass_kernel_spmd(nc, in_maps, list(range(NCORES)), trace=_TRACE)
    if res.exec_time_ns is not None:
        EXEC_TIMES[name] = res.exec_time_ns
    return res.results


# ---------------------------------------------------------------------------
# Launch 1: h1^T = W1^T @ x^T (+ b1), node-sharded
# ---------------------------------------------------------------------------
def _build_l1():
    nc = bass.Bass("TRN2", target_bir_lowering=False, debug=False)
    xt = nc.dram_tensor("xt", [IN_C, NPC_PAD], F32, kind="ExternalInput")
    w1 = nc.dram_tensor("w1", [IN_C, HID], F32, kind="ExternalInput")
    b1 = nc.dram_tensor("b1", [HID, 1], F32, kind="ExternalInput")
    h1t = nc.dram_tensor("h1t", [HID, NPC_PAD], F32, kind="ExternalOutput")

    KK = IN_C // P  # 4 k-tiles
    with TileContext(nc) as tc:
        with (
            tc.tile_pool(name="const", bufs=1) as cpool,
            tc.tile_pool(name="xin", bufs=3) as xpool,
            tc.tile_pool(name="hout", bufs=3) as hpool,
            tc.tile_pool(name="ps", bufs=2, space="PSUM") as pspool,
        ):
            w1_sb = cpool.tile([P, KK * HID], F32, tag="w1")
            for kk in range(KK):
                nc.sync.dma_start(
                    out=w1_sb[:, kk * HID : (kk + 1) * HID],
                    in_=w1[kk * P : (kk + 1) * P, :],
                )
            b1_sb = cpool.tile([HID, 1], F32, tag="b1")
            nc.sync.dma_start(out=b1_sb[:], in_=b1[:, :])

            if True:
                for t in range(TPC):
                    x_sb = xpool.tile([P, KK * P], F32, tag="x")
                    for kk in range(KK):
                        nc.sync.dma_start(
                            out=x_sb[:, kk * P : (kk + 1) * P],
                            in_=xt[kk * P : (kk + 1) * P, t * P : (t + 1) * P],
                        )
                    ps = pspool.tile([HID, P], F32, tag="ps", space="PSUM")
                    for kk in range(KK):
                        nc.tensor.matmul(
                            ps[:],
                            lhsT=w1_sb[:, kk * HID : (kk + 1) * HID],
                            rhs=x_sb[:, kk * P : (kk + 1) * P],
                            start=(kk == 0),
                            stop=(kk == KK - 1),
                        )
                    h_sb = hpool.tile([HID, P], F32, tag="h")
                    nc.vector.tensor_scalar_add(out=h_sb[:], in0=ps[:], scalar1=b1_sb[:, :])
                    nc.gpsimd.dma_start(out=h1t[:, t * P : (t + 1) * P], in_=h_sb[:])
    return nc


# ---------------------------------------------------------------------------
# Launch 2: conv1 — per-edge sigmoid(<h_dst, h_src>), scatter-mean by dst
# ---------------------------------------------------------------------------
def _build_l2(widths, offs, S):
    nc = bass.Bass("TRN2", target_bir_lowering=False, debug=False)
    table = nc.dram_tensor("table", [NTAB_FULL, HID], F32, kind="ExternalInput")
    own_h = nc.dram_tensor("own_h", [NPC_PAD, HID], F32, kind="ExternalInput")
    idx = nc.dram_tensor("idx", [P, S], I32, kind="ExternalInput")
    cnt = nc.dram_tensor("cnt", [P, TPC], F32, kind="ExternalInput")
    uout = nc.dram_tensor("uout", [P, TPC], F32, kind="ExternalOutput")

    with TileContext(nc) as tc:
        with (
            tc.tile_pool(name="const", bufs=1) as cpool,
            tc.tile_pool(name="g", bufs=6) as gpool,
            tc.tile_pool(name="work", bufs=4) as wpool,
            tc.tile_pool(name="colw", bufs=4) as colpool,
        ):
            idx_sb = cpool.tile([P, S], I32, tag="idx")
            nc.sync.dma_start(out=idx_sb[:], in_=idx[:, :])
            cnt_sb = cpool.tile([P, TPC], F32, tag="cnt")
            nc.sync.dma_start(out=cnt_sb[:], in_=cnt[:, :])
            ownh_sb = cpool.tile([P, TPC * HID], F32, tag="ownh")
            for t in range(TPC):
                nc.sync.dma_start(
                    out=ownh_sb[:, t * HID : (t + 1) * HID],
                    in_=own_h[t * P : (t + 1) * P, :],
                )
            u_acc = cpool.tile([P, TPC], F32, tag="uacc")

            for t in range(TPC):
                W = widths[t]
                off = offs[t]
                g = gpool.tile([P, W * HID], F32, tag="g")
                for j in range(W):
                    nc.gpsimd.indirect_dma_start(
                        out=g[:, j * HID : (j + 1) * HID],
                        out_offset=None,
                        in_=table.ap(),
                        in_offset=bass.IndirectOffsetOnAxis(
                            ap=idx_sb[:, off + j : off + j + 1], axis=0
                        ),
                    )
                g_r = g[:].rearrange("p (w k) -> p k w", k=HID)
                hd = ownh_sb[:, t * HID : (t + 1) * HID]
                acc = wpool.tile([P, W], F32, tag="acc")
                tmp = wpool.tile([P, W], F32, tag="tmp")
                nc.vector.tensor_scalar_mul(
                    out=acc[:], in0=g_r[:, 0, :], scalar1=hd[:, 0:1]
                )
                for k in range(1, HID):
                    nc.vector.tensor_scalar_mul(
                        out=tmp[:], in0=g_r[:, k, :], scalar1=hd[:, k : k + 1]
                    )
                    nc.vector.tensor_tensor(
                        out=acc[:], in0=acc[:], in1=tmp[:], op=mybir.AluOpType.add
                    )
                sg = wpool.tile([P, W], F32, tag="sg")
                s_col = colpool.tile([P, 1], F32, tag="scol")
                nc.scalar.activation(
                    out=sg[:],
                    in_=acc[:],
                    func=mybir.ActivationFunctionType.Sigmoid,
                    accum_out=s_col[:],
                )
                # pad slots hit the zero row: sigmoid(0)=0.5 each.
                # s_true = s_col - 0.5*(W - cnt) ; u = relu(s_true / cnt)
                c_col = cnt_sb[:, t : t + 1]
                half_c = colpool.tile([P, 1], F32, tag="halfc")
                nc.vector.tensor_scalar(
                    out=half_c[:],
                    in0=c_col,
                    scalar1=0.5,
                    scalar2=-0.5 * W,
                    op0=mybir.AluOpType.mult,
                    op1=mybir.AluOpType.add,
                )
                s_true = colpool.tile([P, 1], F32, tag="strue")
                nc.vector.tensor_tensor(
                    out=s_true[:], in0=s_col[:], in1=half_c[:], op=mybir.AluOpType.add
                )
                rc = colpool.tile([P, 1], F32, tag="rc")
                nc.vector.reciprocal(out=rc[:], in_=c_col)
                u_col = colpool.tile([P, 1], F32, tag="ucol")
                nc.vector.tensor_tensor(
                    out=u_col[:], in0=s_true[:], in1=rc[:], op=mybir.AluOpType.mult
                )
                nc.vector.tensor_scalar_max(
                    out=u_acc[:, t : t + 1], in0=u_col[:], scalar1=0.0
                )
            nc.gpsimd.dma_start(out=uout[:, :], in_=u_acc[:])
    return nc


# ---------------------------------------------------------------------------
# Launch 3: conv2 on scalar u + final (identically-zero) log_softmax
# ---------------------------------------------------------------------------
def _build_l3(widths, offs, S):
    nc = bass.Bass("TRN2", target_bir_lowering=False, debug=False)
    ut = nc.dram_tensor("ut", [NTAB_FULL, 1], F32, kind="ExternalInput")
    own_u = nc.dram_tensor("own_u", [P, TPC], F32, kind="ExternalInput")
    idx = nc.dram_tensor("idx", [P, S], I32, kind="ExternalInput")
    cnt = nc.dram_tensor("cnt", [P, TPC], F32, kind="ExternalInput")
    w2r = nc.dram_tensor("w2r", [P, OUT_C], F32, kind="ExternalInput")
    b2r = nc.dram_tensor("b2r", [P, OUT_C], F32, kind="ExternalInput")
    zout = nc.dram_tensor("zout", [P, TPC], F32, kind="ExternalOutput")

    with TileContext(nc) as tc:
        with (
            tc.tile_pool(name="const", bufs=1) as cpool,
            tc.tile_pool(name="g", bufs=6) as gpool,
            tc.tile_pool(name="work", bufs=4) as wpool,
            tc.tile_pool(name="colw", bufs=6) as colpool,
        ):
            idx_sb = cpool.tile([P, S], I32, tag="idx")
            nc.sync.dma_start(out=idx_sb[:], in_=idx[:, :])
            cnt_sb = cpool.tile([P, TPC], F32, tag="cnt")
            nc.sync.dma_start(out=cnt_sb[:], in_=cnt[:, :])
            ownu_sb = cpool.tile([P, TPC], F32, tag="ownu")
            nc.sync.dma_start(out=ownu_sb[:], in_=own_u[:, :])
            w2_sb = cpool.tile([P, OUT_C], F32, tag="w2")
            nc.sync.dma_start(out=w2_sb[:], in_=w2r[:, :])
            b2_sb = cpool.tile([P, OUT_C], F32, tag="b2")
            nc.sync.dma_start(out=b2_sb[:], in_=b2r[:, :])

            # a = |W2|^2, b = <W2,b2>, c = |b2|^2  (per-partition copies)
            sq = cpool.tile([P, OUT_C], F32, tag="sq")
            a_col = cpool.tile([P, 1], F32, tag="acol")
            b_col = cpool.tile([P, 1], F32, tag="bcol")
            c_col = cpool.tile([P, 1], F32, tag="ccol")
            nc.vector.tensor_tensor(
                out=sq[:], in0=w2_sb[:], in1=w2_sb[:], op=mybir.AluOpType.mult
            )
            nc.vector.tensor_reduce(
                out=a_col[:], in_=sq[:], axis=mybir.AxisListType.X, op=mybir.AluOpType.add
            )
            nc.vector.tensor_tensor(
                out=sq[:], in0=w2_sb[:], in1=b2_sb[:], op=mybir.AluOpType.mult
            )
            nc.vector.tensor_reduce(
                out=b_col[:], in_=sq[:], axis=mybir.AxisListType.X, op=mybir.AluOpType.add
            )
            nc.vector.tensor_tensor(
                out=sq[:], in0=b2_sb[:], in1=b2_sb[:], op=mybir.AluOpType.mult
            )
            nc.vector.tensor_reduce(
                out=c_col[:], in_=sq[:], axis=mybir.AxisListType.X, op=mybir.AluOpType.add
            )

            z_acc = cpool.tile([P, TPC], F32, tag="zacc")

            for t in range(TPC):
                W = widths[t]
                off = offs[t]
                g = gpool.tile([P, W], F32, tag="g")
                for j in range(W):
                    nc.gpsimd.indirect_dma_start(
                        out=g[:, j : j + 1],
                        out_offset=None,
                        in_=ut.ap(),
                        in_offset=bass.IndirectOffsetOnAxis(
                            ap=idx_sb[:, off + j : off + j + 1], axis=0
                        ),
                    )
                ud = ownu_sb[:, t : t + 1]
                # lin_d = b*u_d + c ; coef_d = a*u_d + b (per-node columns)
                lin_d = colpool.tile([P, 1], F32, tag="lind")
                nc.vector.tensor_scalar(
                    out=lin_d[:], in0=ud, scalar1=b_col[:, :], scalar2=c_col[:, :],
                    op0=mybir.AluOpType.mult, op1=mybir.AluOpType.add,
                )
                coef_d = colpool.tile([P, 1], F32, tag="coefd")
                nc.vector.tensor_scalar(
                    out=coef_d[:], in0=ud, scalar1=a_col[:, :], scalar2=b_col[:, :],
                    op0=mybir.AluOpType.mult, op1=mybir.AluOpType.add,
                )
                # arg = coef_d * u_s + lin_d
                arg = wpool.tile([P, W], F32, tag="arg")
                nc.vector.tensor_scalar(
                    out=arg[:], in0=g[:], scalar1=coef_d[:, :], scalar2=lin_d[:, :],
                    op0=mybir.AluOpType.mult, op1=mybir.AluOpType.add,
                )
                sg = wpool.tile([P, W], F32, tag="sg")
                s_col = colpool.tile([P, 1], F32, tag="scol")
                nc.scalar.activation(
                    out=sg[:], in_=arg[:],
                    func=mybir.ActivationFunctionType.Sigmoid,
                    accum_out=s_col[:],
                )
                # pad slots: u_s = 0 -> sigmoid(lin_d) each; subtract (W-cnt)*sigmoid(lin_d)
                sg_pad = colpool.tile([P, 1], F32, tag="sgpad")
                nc.scalar.activation(
                    out=sg_pad[:], in_=lin_d[:],
                    func=mybir.ActivationFunctionType.Sigmoid,
                )
                cc = cnt_sb[:, t : t + 1]
                wmc = colpool.tile([P, 1], F32, tag="wmc")
                nc.vector.tensor_scalar(
                    out=wmc[:], in0=cc, scalar1=-1.0, scalar2=float(W),
                    op0=mybir.AluOpType.mult, op1=mybir.AluOpType.add,
                )
                corr = colpool.tile([P, 1], F32, tag="corr")
                nc.vector.tensor_tensor(
                    out=corr[:], in0=wmc[:], in1=sg_pad[:], op=mybir.AluOpType.mult
                )
                s_true = colpool.tile([P, 1], F32, tag="strue")
                nc.vector.tensor_tensor(
                    out=s_true[:], in0=s_col[:], in1=corr[:],
                    op=mybir.AluOpType.subtract,
                )
                rc = colpool.tile([P, 1], F32, tag="rc")
                nc.vector.reciprocal(out=rc[:], in_=cc)
                v_col = colpool.tile([P, 1], F32, tag="vcol")
                nc.vector.tensor_tensor(
                    out=v_col[:], in0=s_true[:], in1=rc[:], op=mybir.AluOpType.mult
                )
                # log_softmax over a single-element axis: v - logsumexp(v) = v - v
                nc.vector.tensor_tensor(
                    out=z_acc[:, t : t + 1], in0=v_col[:], in1=v_col[:],
                    op=mybir.AluOpType.subtract,
                )
            nc.gpsimd.dma_start(out=zout[:, :], in_=z_acc[:])
    return nc


# ---------------------------------------------------------------------------
# Host orchestration
# ---------------------------------------------------------------------------
def kernel(x, edge_index, W1, b1, W2, b2):
    x = np.asarray(x, dtype=np.float32)
    edge_index = np.asarray(edge_index)
    W1 = np.asarray(W1, dtype=np.float32)
    b1 = np.asarray(b1, dtype=np.float32)
    W2 = np.asarray(W2, dtype=np.float32)
    b2 = np.asarray(b2, dtype=np.float32)
    assert x.shape == (N, IN_C)

    # ---- structural preprocessing (host) ----
    src = np.concatenate([edge_index[0], np.arange(N, dtype=np.int64)])
    dst = np.concatenate([edge_index[1], np.arange(N, dtype=np.int64)])
    deg = np.bincount(dst, minlength=N).astype(np.int64)  # includes self-loop

    core_of = (dst // NPC).astype(np.int64)

    # per-core node ordering: degree-descending within the core's node range
    perm = np.empty((NCORES, NPC), dtype=np.int64)  # perm[k, slot] = orig node id
    posmap = np.empty(N, dtype=np.int64)            # orig id -> global permuted id
    for k in range(NCORES):
        ids = np.arange(k * NPC, (k + 1) * NPC)
        order = np.argsort(-deg[ids], kind="stable")
        perm[k] = ids[order]
        posmap[perm[k]] = k * NPC_PAD + np.arange(NPC)

    src_p = posmap[src]

    # tile widths: max over cores of per-tile max degree (shared SPMD schedule)
    deg_pad = np.zeros((NCORES, NPC_PAD), dtype=np.int64)
    for k in range(NCORES):
        deg_pad[k, :NPC] = deg[perm[k]]
    deg_tiles = deg_pad.reshape(NCORES, TPC, P)
    widths = deg_tiles.max(axis=(0, 2)).astype(np.int64)
    widths = np.maximum(widths, 1)
    offs = np.concatenate([[0], np.cumsum(widths)[:-1]]).astype(np.int64)
    S = int(widths.sum())

    # ELL packing per core: idx_big[core, p, off_t + j] = permuted src id
    idx_big = np.full((NCORES, P, S), ZROW, dtype=np.int32)
    cnt_arr = np.ones((NCORES, P, TPC), dtype=np.float32)
    Wmax = int(widths.max())
    for k in range(NCORES):
        m = core_of[src.shape[0] * 0 :] == k  # noqa: keep full mask
        m = core_of == k
        d_k = dst[m]
        s_k = src_p[m]
        slot = posmap[d_k] - k * NPC_PAD  # 0..NPC-1
        order = np.argsort(slot, kind="stable")
        slot = slot[order]
        s_k = s_k[order]
        counts = np.bincount(slot, minlength=NPC_PAD)
        row_ptr = np.concatenate([[0], np.cumsum(counts)])
        rank = np.arange(slot.shape[0]) - row_ptr[slot]
        ell = np.full((NPC_PAD, Wmax), ZROW, dtype=np.int64)
        ell[slot, rank] = s_k
        ell_t = ell.reshape(TPC, P, Wmax)
        for t in range(TPC):
            W = int(widths[t])
            idx_big[k, :, offs[t] : offs[t] + W] = ell_t[t, :, :W].astype(np.int32)
        cnt_arr[k][:, :] = np.maximum(
            deg_pad[k].reshape(TPC, P).T.astype(np.float32), 1.0
        )

    # ---- launch 1: h1 ----
    xt_full = np.ascontiguousarray(x.T)  # [512, N]
    l1 = _build_l1()
    in_maps = []
    for k in range(NCORES):
        xt_k = np.zeros((IN_C, NPC_PAD), dtype=np.float32)
        xt_k[:, :NPC] = xt_full[:, perm[k]]
        in_maps.append({"xt": xt_k, "w1": W1, "b1": b1.reshape(HID, 1)})
    outs = _run("l1", l1, in_maps)

    table = np.zeros((NTAB_FULL, HID), dtype=np.float32)
    for k in range(NCORES):
        table[k * NPC_PAD : (k + 1) * NPC_PAD] = outs[k]["h1t"].T
    # pad node slots (beyond NPC) computed h1 = b1 (x=0); they are never
    # referenced as src. Zero rows at ZROW.. serve pad gather slots.
    table[ZROW:] = 0.0

    # ---- launch 2: conv1 ----
    l2 = _build_l2(widths, offs, S)
    in_maps = []
    for k in range(NCORES):
        in_maps.append(
            {
                "table": table,
                "own_h": table[k * NPC_PAD : (k + 1) * NPC_PAD],
                "idx": idx_big[k],
                "cnt": cnt_arr[k],
            }
        )
    outs = _run("l2", l2, in_maps)

    u_tab = np.zeros((NTAB_FULL, 1), dtype=np.float32)
    own_u = []
    for k in range(NCORES):
        u_k = outs[k]["uout"]  # [P, TPC] : node (t,p) at [p,t]
        own_u.append(u_k)
        u_tab[k * NPC_PAD : (k + 1) * NPC_PAD, 0] = u_k.T.reshape(-1)
    u_tab[ZROW:] = 0.0
    # pad node slots (>=NPC within a core) have junk-finite u; they are never
    # gathered (no edges reference them), but zero them for cleanliness:
    for k in range(NCORES):
        u_tab[k * NPC_PAD + NPC : (k + 1) * NPC_PAD] = 0.0

    # ---- launch 3: conv2 + log_softmax ----
    l3 = _build_l3(widths, offs, S)
    w2r = np.tile(W2.reshape(1, OUT_C), (P, 1)).astype(np.float32)
    b2r = np.tile(b2.reshape(1, OUT_C), (P, 1)).astype(np.float32)
    in_maps = []
    for k in range(NCORES):
        in_maps.append(
            {
                "ut": u_tab,
                "own_u": own_u[k],
                "idx": idx_big[k],
                "cnt": cnt_arr[k],
                "w2r": w2r,
                "b2r": b2r,
            }
        )
    outs = _run("l3", l3, in_maps)

    # ---- reassemble full output (inverse permutation) ----
    result = np.zeros((N, 1), dtype=np.float32)
    for k in range(NCORES):
        z_k = outs[k]["zout"]  # [P, TPC]
        z_flat = z_k.T.reshape(-1)[:NPC]  # node slot order
        result[perm[k], 0] = z_flat
    return result
